# revision 25
# baseline (speedup 1.0000x reference)
"""Trainium2 Bass kernel for nn_BandwidthPredictorNNHall.

Math: for each batch b (8 of them, one per NeuronCore) with particles
x [n=1024, d=4]:
    pilot_d = 1.0592 * std(x_d, ddof=1) * n^(-1/8)
    q = x / pilot,   K_ij = exp(-0.5 * |q_i - q_j|^2)
    s2_d = sum_ij K_ij ((q_jd - q_id)^2 - 1)
    s3_d = sum_ij K_ij (dx^3 - 3 dx)  == 0 exactly (odd under i<->j swap),
           so bandwidth2 is fp-cancellation noise in the reference
           (|bw2/bw1| ~ 6e-9) and is treated as 0.
With Mp = [1, p_1..p_4, p_1^2..p_4^2] (n x 9, RAW particle units), every sum
needed for s2 is an entry of V = Mp^T K Mp after a host-side 1/pilot^2
rescale:
    s2_d = ((V[0,5+d] + V[5+d,0] - 2 V[1+d,1+d]) / pilot_d^2 - V[0,0]) / sqrt(2pi)
The device computes V (9x9) and var (4) per batch; the host applies the
final ~30 scalar flops per batch.

Device pipeline per core (engine-balance driven; ScalarE's 1M exps are the
floor, everything else hides behind or around them):
  - One input DMA (each dma_start costs ~0.6us of queue time plus ~1.5us
    latency): a 3D-strided load mstatall [128, 8(tile), 4] in particle-major
    layout. The feature-major Gram operands are built from it with 8 small
    PE transposes instead of a second (slow, 4-byte-run) strided DMA.
  - sum(p) and sum(p^2) accumulate on the PE as two sequential matmul
    groups against a ones vector; tiny PE transposes move the results into
    row form. var/pinv2 = 1/(FACT^2 var) needs only a reciprocal -- no
    sqrt, so ScalarE runs just {Exp, Copy}: one activation-table set, one
    LoadActFuncSet, and it overlaps the DMA latency.
  - G_ij = q_i . q_j is computed as sum_d (p_id/pilot_d^2) p_jd (float32r
    streams at 1 cycle/row; even bf16-coarse rounding would move the final
    output < 1.5e-4, far below the reference's own fp32 noise).
  - K'' = exp(G - r_i/2): one [128,1024] ScalarE activation per row tile
    with per-partition bias, reading a 2-bank PSUM tile. r_i comes from a
    multiply + negated reduce against a PE-broadcast 0.5/pilot^2 row.
  - K'' is the true K column-scaled by c_j = e^{+r_j/2}; the scale is
    constant per column so it factors through P = K M and is cancelled
    exactly in stage V by MX = Mp . e^{-r/2}:
        PT  = Mp^T K''   (9-column weight loads, f32r stream, two PSUM
                          accumulation groups that chase the exp stream)
        P'' = PT^T per 128-col block (8 small PE transposes, one PSUM bank)
        V   = MX^T P'' = Mp^T K Mp
  - K symmetry makes the stored K'' row-tiles serve both orientations, so
    the [n,n] matrix is never transposed.
"""

import sys

sys.path.insert(0, "/opt/trn_rl_repo")

import numpy as np

_B, _N, _D = 8, 1024, 4
_P = 128
_NT = _N // _P  # 8 row tiles
_NM = 1 + 2 * _D  # 9 basis columns: [1, p, p^2]
_INV_SQRT_2PI = 1.0 / np.sqrt(2.0 * np.pi)
_RK = 0.282095
_FACT = 1.0592 * float(_N) ** (-1.0 / (4 + _D))

_NC = None  # compiled Bass module cache


def _build_kernel():
    import concourse.bass as bass  # noqa: F401
    import concourse.tile as tile
    from concourse import bacc, mybir
    from concourse.masks import make_identity

    f32 = mybir.dt.float32
    fr = mybir.dt.float32r
    Act = mybir.ActivationFunctionType
    Alu = mybir.AluOpType
    Ax = mybir.AxisListType

    nc = bacc.Bacc("TRN2", target_bir_lowering=False, debug=False, num_devices=_B)
    p_in = nc.dram_tensor("p", [_N, _D], f32, kind="ExternalInput")
    v_out = nc.dram_tensor("vout", [_NM, _NM], f32, kind="ExternalOutput")
    var_out = nc.dram_tensor("varout", [_D, 1], f32, kind="ExternalOutput")

    with tile.TileContext(nc) as tc:
        with (
            tc.tile_pool(name="singles", bufs=1) as singles,
            tc.tile_pool(name="psE", bufs=1, space="PSUM") as psE,
            tc.tile_pool(name="psV", bufs=1, space="PSUM") as psV,
            tc.tile_pool(name="psG", bufs=2, space="PSUM") as psG,
            tc.tile_pool(name="psPT", bufs=1, space="PSUM") as psPT,
        ):
            ident128 = singles.tile([_P, _P], f32, tag="identf")
            make_identity(nc, ident128)
            ident = ident128[0:_NM, 0:_NM]
            ones128 = singles.tile([_P, 1], f32, tag="ones128")
            nc.vector.memset(ones128, 1.0)
            ones_row = singles.tile([1, _P], f32, tag="ones_row")
            nc.vector.memset(ones_row, 1.0)
            onesN = singles.tile([_P, 1], f32, tag="onesN")
            nc.vector.memset(onesN, 1.0 / float(_N) ** 0.5)
            # dummy Exp so the activation-table load runs during the DMA wait
            warm = singles.tile([1, 1], f32, tag="warm")
            nc.scalar.activation(out=warm, in_=ones128[0:1, 0:1], func=Act.Exp)

            # ---- two input DMAs: particle-major tiles + feature-major rows
            mstatall = singles.tile([_P, _NT, _D], f32, tag="mstatall")
            nc.sync.dma_start(
                out=mstatall, in_=p_in[:].rearrange("(c i) d -> i c d", c=_NT)
            )
            msqall = singles.tile([_P, _NT, _D], f32, tag="msqall")
            nc.vector.tensor_mul(msqall, mstatall, mstatall)

            # ---- stats on the PE: two sequential accumulation groups
            # (sum p, then sum p^2), each copied out and transposed to a
            # row so the var chain runs at partition 0
            sv4 = []
            for g, (src, rv) in enumerate(((mstatall, onesN), (msqall, ones128))):
                st4 = psE.tile([_D, 1], f32, tag="early")
                for c in range(_NT):
                    nc.tensor.matmul(
                        st4, lhsT=src[:, c, :], rhs=rv,
                        start=(c == 0), stop=(c == _NT - 1),
                    )
                sv = singles.tile([_D, 1], f32, tag=f"sv4_{g}")
                nc.vector.tensor_copy(sv, st4)
                sv4.append(sv)
            # den = sum(p^2) - sum(p)^2/n = (n-1) var; phcol = 0.5/pilot^2
            den = singles.tile([_D, 1], f32, tag="den")
            nc.vector.tensor_mul(den, sv4[0], sv4[0])
            nc.vector.tensor_sub(den, sv4[1], den)
            var_t = singles.tile([_D, 1], f32, tag="var_t")
            nc.vector.tensor_scalar_mul(var_t, den, 1.0 / (_N - 1))
            nc.sync.dma_start(out=var_out[:], in_=var_t)
            denf = singles.tile([_D, 1], f32, tag="denf")
            nc.vector.tensor_scalar_mul(
                denf, den, 2.0 * _FACT * _FACT / (_N - 1)
            )
            phcol = singles.tile([_D, 1], f32, tag="phcol")
            nc.vector.reciprocal(phcol, denf)

            # QTr = p in feature-major f32r via 8 PE transposes of the
            # tile-major data (no second DMA); Qs = QTr * 2*phcol
            QTr = singles.tile([_D, _N], fr, tag="qtr")
            for c in range(_NT):
                cs = slice(c * _P, (c + 1) * _P)
                ps_q = psG.tile([_D, _P], f32, tag="psg")
                nc.tensor.transpose(ps_q, mstatall[:, c, :], ident128)
                nc.vector.tensor_copy(QTr[:, cs], ps_q)
            Qs = singles.tile([_D, _N], fr, tag="qs")
            nc.vector.tensor_scalar(
                out=Qs, in0=QTr, scalar1=phcol, scalar2=2.0,
                op0=Alu.mult, op1=Alu.mult,
            )

            # 0.5/pilot^2 as a row + broadcast to [128,4] via rank-1 PE
            # outer product (for the r_i reductions)
            ps_pr = psE.tile([1, _D], f32, tag="early")
            nc.tensor.transpose(ps_pr, phcol, ident[0:_D, 0:_D])
            ph_r = singles.tile([1, _D], f32, tag="ph_r")
            nc.vector.tensor_copy(ph_r, ps_pr)
            ps_bc = psE.tile([_P, _D], f32, tag="early")
            nc.tensor.matmul(ps_bc, lhsT=ones_row, rhs=ph_r, start=True, stop=True)
            bc_sb = singles.tile([_P, _D], f32, tag="bc_sb")
            nc.vector.tensor_copy(bc_sb, ps_bc)

            # ---- exp bias nhall[:, c] = -r/2 = -sum_d p^2 * (0.5/pilot^2)
            nhall = singles.tile([_P, _NT], f32, tag="nhall")
            scr = singles.tile([_P, _NT, _D], f32, tag="scr")
            for c in range(_NT):
                nc.vector.tensor_mul(scr[:, c, :], msqall[:, c, :], bc_sb)
                nc.vector.tensor_reduce(
                    out=nhall[:, c : c + 1], in_=scr[:, c, :],
                    axis=Ax.X, op=Alu.add, negate=True,
                )

            # ---- Mp tiles (f32r, PT-stage weights) built in strided copies;
            # MX = Mp . e^{-r/2} per tile
            mtall = singles.tile([_P, _NT, _NM], fr, tag="mtall")
            for c in range(_NT):
                nc.vector.tensor_copy(mtall[:, c, 0:1], ones128)
            nc.vector.tensor_copy(mtall[:, :, 1 : 1 + _D], mstatall)
            nc.vector.tensor_copy(mtall[:, :, 1 + _D : _NM], msqall)
            cneg = singles.tile([_P, _NT], f32, tag="cneg")
            nc.scalar.activation(out=cneg, in_=nhall, func=Act.Exp)
            mxall = singles.tile([_P, _NT, _NM], f32, tag="mxall")
            for c in range(_NT):
                nc.vector.tensor_scalar_mul(
                    mxall[:, c, :], mtall[:, c, :], cneg[:, c : c + 1]
                )

            # ---- main stream: per row tile, two f32r Gram matmuls into a
            # 2-bank PSUM tile, one [128,1024] Exp, then the tile's PT
            # contributions (both j-half accumulation groups chase the exps)
            KT = singles.tile([_P, _NT, _N], fr, tag="kt")
            pspt = psPT.tile([_NM, 2, 512], f32, tag="pspt")
            for ir in range(_NT):
                irs = slice(ir * _P, (ir + 1) * _P)
                psg = psG.tile([_P, 2, 512], f32, tag="psg")
                for jh in range(2):
                    js = slice(jh * 512, (jh + 1) * 512)
                    nc.tensor.matmul(
                        psg[:, jh, :],
                        lhsT=Qs[:, irs],
                        rhs=QTr[:, js],
                        start=True, stop=True,
                    )
                nc.scalar.activation(
                    out=KT[:, ir, :],
                    in_=psg.rearrange("p a b -> p (a b)"),
                    func=Act.Exp,
                    bias=nhall[:, ir : ir + 1],
                )
                for jh in range(2):
                    js = slice(jh * 512, (jh + 1) * 512)
                    nc.tensor.matmul(
                        pspt[:, jh, :],
                        lhsT=mtall[:, ir, :],
                        rhs=KT[:, ir, js],
                        start=(ir == 0), stop=(ir == _NT - 1),
                        skip_group_check=True,
                    )

            # ---- PT out of PSUM, P'' = PT^T per block into one PSUM bank,
            # V = MX^T P''
            pts = singles.tile([_NM, _N], f32, tag="pts")
            nc.vector.tensor_copy(pts[:, 0:512], pspt[:, 0, :])
            nc.vector.tensor_copy(pts[:, 512:1024], pspt[:, 1, :])
            psp2 = psE.tile([_P, _NT, _NM], f32, tag="early")
            for r in range(_NT):
                nc.tensor.transpose(
                    psp2[:, r, :], pts[:, r * _P : (r + 1) * _P], ident
                )
            prall = singles.tile([_P, _NT, _NM], f32, tag="prall")
            nc.vector.tensor_copy(prall, psp2)
            psv = psV.tile([_NM, _NM], f32, tag="psv")
            for r in range(_NT):
                nc.tensor.matmul(
                    psv, lhsT=mxall[:, r, :], rhs=prall[:, r, :],
                    start=(r == 0), stop=(r == _NT - 1),
                )
            Vt = singles.tile([_NM, _NM], f32, tag="vt")
            nc.vector.tensor_copy(Vt, psv)
            nc.sync.dma_start(out=v_out[:], in_=Vt)

    nc.compile()
    return nc


def _get_nc():
    global _NC
    if _NC is None:
        _NC = _build_kernel()
    return _NC


def finalize(V, var):
    """Host-side tail: V [9,9] (raw-p units), var [4] -> bandwidth [4]."""
    V = V.astype(np.float64)
    var = var.astype(np.float64).reshape(_D)
    pilot = _FACT * np.sqrt(var)
    d = np.arange(_D)
    s2 = (
        (V[0, 5 + d] + V[5 + d, 0] - 2.0 * V[1 + d, 1 + d]) / pilot**2 - V[0, 0]
    ) * _INV_SQRT_2PI
    denom = _N * (_N - 1)
    I2 = s2 / pilot**5 / denom
    J1 = _RK / I2
    base = J1 / _N
    return (np.sign(base) * np.abs(base) ** 0.2).astype(np.float32)


def kernel(particles, weights=None, **_unused):
    from concourse.bass_utils import run_bass_kernel_spmd

    particles = np.ascontiguousarray(np.asarray(particles), dtype=np.float32)
    assert particles.shape == (_B, _N, _D), particles.shape

    nc = _get_nc()
    in_maps = [{"p": particles[c]} for c in range(_B)]
    res = run_bass_kernel_spmd(nc, in_maps, list(range(_B)))

    out = np.empty((_B, _D), np.float32)
    for c in range(_B):
        out[c] = finalize(res.results[c]["vout"], res.results[c]["varout"])
    return out
